# revision 5
# baseline (speedup 1.0000x reference)
"""Trainium2 Bass kernel for batched multi-head attention.

Problem: B=8, H=8, S=2048, D=64 f32 attention,
  out = softmax(Q K^T / 64**0.25) V  per (b, h).

Sharding: the 64 (b,h) pairs are split 8-per-core across the 8 NeuronCores
(pure data/head parallelism, no collectives).

v2 design (vs v1 baseline at ~315us):
  - The v1 kernel was ACT-bound: all 33.6M softmax exps per core ran on the
    Scalar engine (~260us busy).  v2 splits exp across engines: 5/8 of the
    k-chunk pairs use exact ACT exp, 3/8 use a Schraudolph-style fast exp on
    the (previously idle) Vector engine - one tensor_scalar op computing
    int16(bf16-bits) = scores*A + B, bit-reinterpreted as bf16 weights
    (rel-err ~1e-2 end to end, tolerance 2e-2).
  - QK strip pairs (K=64 contraction) get explicit tile_position row tiling
    so both 64-row matmuls stream concurrently.
  - Issue order software-pipelines one (head, slab) stage: QK of stage i+1
    issues before AV of stage i, so the exp engines never wait on the PE
    FIFO draining AV before producing the next slab's scores.
  - Softmax denominators come from a ones-column in the AV stationary
    (PSUM row 64); per-slab they bounce through DRAM into a [64, 512]
    broadcast and fold into the single normalize tensor_tensor multiply
    (no separate PSUM->SBUF copy).
"""
import sys

sys.path.insert(0, "/opt/trn_rl_repo")

from contextlib import ExitStack

import ml_dtypes
import numpy as np

import concourse.bass as bass
import concourse.tile as tile
from concourse import bacc, mybir
from concourse.bass_utils import run_bass_kernel_spmd

B, H, S, D = 8, 8, 2048, 64
N_CORES = 8
HPC = B * H // N_CORES  # heads per core = 8
SCALE = 1.0 / (D**0.5) ** 0.5  # 1 / 64**0.25
PCHUNK = 128  # k rows per chunk
NCHUNK = S // PCHUNK  # 16
NPAIR = NCHUNK // 2  # 8 chunk pairs per slab
SLAB = 512  # q columns per slab
NSLAB = S // SLAB  # 4
BF16 = mybir.dt.bfloat16
F32 = mybir.dt.float32
I16 = mybir.dt.int16

# Schraudolph fast-exp on DVE: bf16 bits of exp(s*SCALE) ~= s*A1 + B1
LOG2E = 1.4426950408889634
FEXP_C = 0.06  # sawtooth centering, tuned in numerics_sim.py
A1 = 128.0 * LOG2E * SCALE
B1 = 128.0 * (127.0 - FEXP_C)
DVE_PAIRS = (2, 5, 7)  # pairs computed on DVE; rest on ACT

_COMPILED = {}


def build_kernel():
    nc = bacc.Bacc("TRN2", target_bir_lowering=False, debug=False)
    qt = nc.dram_tensor("q_t", [HPC, D, S], BF16, kind="ExternalInput").ap()
    kt = nc.dram_tensor("k_t", [HPC, D, S], BF16, kind="ExternalInput").ap()
    v = nc.dram_tensor("v", [HPC, S, D], BF16, kind="ExternalInput").ap()
    out = nc.dram_tensor("out_t", [HPC, D, S], F32, kind="ExternalOutput").ap()
    # DRAM bounce buffers for the cross-partition softmax-denominator move
    s_dram = nc.dram_tensor("s_scratch", [HPC, NSLAB, SLAB], F32).ap()
    r_dram = nc.dram_tensor("r_scratch", [HPC, NSLAB, SLAB], F32).ap()

    with tile.TileContext(nc) as tc, ExitStack() as ctx:
        qk_pool = ctx.enter_context(tc.tile_pool(name="qk", bufs=2))
        v_pool = ctx.enter_context(tc.tile_pool(name="vp", bufs=2))
        exp_pool = ctx.enter_context(tc.tile_pool(name="exp", bufs=2))
        fin_pool = ctx.enter_context(tc.tile_pool(name="fin", bufs=3))
        small_pool = ctx.enter_context(tc.tile_pool(name="small", bufs=3))
        const_pool = ctx.enter_context(tc.tile_pool(name="const", bufs=1))
        # PSUM: psqk 3 x 2 banks + psav 2 x 1 bank = 8 banks exactly
        psqk_pool = ctx.enter_context(
            tc.tile_pool(name="psqk", bufs=3, space="PSUM")
        )
        psav_pool = ctx.enter_context(
            tc.tile_pool(name="psav", bufs=2, space="PSUM")
        )

        zbias = const_pool.tile([128, 1], F32)
        nc.vector.memset(zbias[:], 0.0)
        # warm the ACT exp table at t=0 so its ~2.7us load overlaps the first
        # input DMAs instead of delaying the first real exp
        warm = const_pool.tile([128, 1], F32)
        nc.scalar.activation(
            warm[:],
            zbias[:],
            mybir.ActivationFunctionType.Exp,
            bias=zbias[:],
            scale=1.0,
        )

        head_tiles = {}

        def load_head(h):
            qt_sb = qk_pool.tile([2 * D, S], BF16, tag="qt")
            kt_sb = qk_pool.tile([2 * D, S], BF16, tag="kt")
            HS = S // 2
            for piece in range(2):
                cols = slice(piece * HS, (piece + 1) * HS)
                nc.sync.dma_start(kt_sb[0:D, cols], kt[h][:, cols])
                nc.sync.dma_start(kt_sb[D : 2 * D, cols], kt[h][:, cols])
                nc.sync.dma_start(qt_sb[0:D, cols], qt[h][:, cols])
                nc.sync.dma_start(qt_sb[D : 2 * D, cols], qt[h][:, cols])
            v_aug = v_pool.tile([PCHUNK, NCHUNK, D + 1], BF16, tag="vaug")
            nc.sync.dma_start(
                v_aug[:, :, 0:D], v[h].rearrange("(c p) d -> p c d", p=PCHUNK)
            )
            nc.vector.memset(v_aug[:, :, D : D + 1], 1.0)
            head_tiles[h] = (qt_sb, kt_sb, v_aug)

        def issue_qk(h, s):
            qt_sb, kt_sb, _ = head_tiles[h]
            expT = exp_pool.tile([PCHUNK, NCHUNK, SLAB], BF16, tag="expT")
            cols = slice(s * SLAB, (s + 1) * SLAB)
            for p in range(NPAIR):
                ps = psqk_pool.tile([PCHUNK, 2, SLAB], F32, tag="psqk")
                for half in range(2):
                    c = 2 * p + half
                    base = half * D
                    nc.tensor.matmul(
                        ps[:, half, :],
                        kt_sb[base : base + D, c * PCHUNK : (c + 1) * PCHUNK],
                        qt_sb[base : base + D, cols],
                        start=True,
                        stop=True,
                        tile_position=(base, 0),
                    )
                dst = expT[:, 2 * p : 2 * p + 2, :]
                if p in DVE_PAIRS:
                    nc.vector.tensor_scalar(
                        dst.bitcast(I16),
                        ps[:],
                        A1,
                        B1,
                        mybir.AluOpType.mult,
                        mybir.AluOpType.add,
                    )
                else:
                    nc.scalar.activation(
                        dst,
                        ps[:],
                        mybir.ActivationFunctionType.Exp,
                        bias=zbias[:],
                        scale=SCALE,
                    )
            return expT

        def issue_av(h, s, expT):
            _, _, v_aug = head_tiles[h]
            po = psav_pool.tile([D + 1, SLAB], F32, tag="psav")
            for c in range(NCHUNK):
                nc.tensor.matmul(
                    po[:],
                    v_aug[:, c, :],
                    expT[:, c, :],
                    start=(c == 0),
                    stop=(c == NCHUNK - 1),
                )
            # softmax denominators for this slab: PSUM row 64 -> SBUF (DMA
            # cannot read PSUM) -> DRAM -> [128, SLAB/128] tile ->
            # reciprocal -> DRAM -> [D, SLAB] stride-0 broadcast
            row_sb = small_pool.tile([1, SLAB], F32, tag="rowsb")
            nc.vector.tensor_copy(row_sb[:], po[D : D + 1, :])
            nc.sync.dma_start(s_dram[h, s], row_sb[:])
            sums = small_pool.tile([128, SLAB // 128], F32, tag="sums")
            nc.sync.dma_start(
                sums[:], s_dram[h, s].rearrange("(c p) -> p c", p=128)
            )
            rnat = small_pool.tile([128, SLAB // 128], F32, tag="rnat")
            nc.vector.reciprocal(rnat[:], sums[:])
            nc.sync.dma_start(
                r_dram[h, s].rearrange("(c p) -> p c", p=128), rnat[:]
            )
            r_bc = small_pool.tile([D, SLAB], F32, tag="rbc")
            nc.sync.dma_start(
                r_bc[:],
                bass.AP(r_dram.tensor, (h * NSLAB + s) * SLAB, [[0, D], [1, SLAB]]),
            )
            o_fin = fin_pool.tile([D, SLAB], F32, tag="ofin")
            nc.vector.tensor_tensor(
                o_fin[:], po[0:D, :], r_bc[:], op=mybir.AluOpType.mult
            )
            nc.sync.dma_start(out[h][:, s * SLAB : (s + 1) * SLAB], o_fin[:])

        stages = [(h, s) for h in range(HPC) for s in range(NSLAB)]
        pending = None  # (h, s, expT) whose AV hasn't issued yet
        for h, s in stages:
            if s == 0:
                load_head(h)
            expT = issue_qk(h, s)
            if pending is not None:
                issue_av(*pending)
            pending = (h, s, expT)
        issue_av(*pending)
    nc.compile()
    return nc


def _get_compiled():
    if "nc" not in _COMPILED:
        _COMPILED["nc"] = build_kernel()
    return _COMPILED["nc"]


def kernel(query, key, value, _want_results=False):
    nc = _get_compiled()
    q = np.asarray(query).reshape(B * H, S, D)
    k = np.asarray(key).reshape(B * H, S, D)
    v = np.asarray(value).reshape(B * H, S, D)
    in_maps = []
    for c in range(N_CORES):
        sl = slice(c * HPC, (c + 1) * HPC)
        in_maps.append(
            {
                "q_t": np.ascontiguousarray(q[sl].transpose(0, 2, 1)).astype(
                    ml_dtypes.bfloat16
                ),
                "k_t": np.ascontiguousarray(k[sl].transpose(0, 2, 1)).astype(
                    ml_dtypes.bfloat16
                ),
                "v": np.ascontiguousarray(v[sl]).astype(ml_dtypes.bfloat16),
            }
        )
    res = run_bass_kernel_spmd(nc, in_maps, core_ids=list(range(N_CORES)))
    out = np.concatenate(
        [
            res.results[c]["out_t"].transpose(0, 2, 1).reshape(1, HPC, S, D)
            for c in range(N_CORES)
        ],
        axis=0,
    ).reshape(B, H, S, D)
    if _want_results:
        return out, res
    return out


if __name__ == "__main__":
    rng = np.random.default_rng(0)
    q = rng.standard_normal((B, H, S, D), dtype=np.float32)
    k = rng.standard_normal((B, H, S, D), dtype=np.float32)
    v = rng.standard_normal((B, H, S, D), dtype=np.float32)
    o = kernel(q, k, v)
    print("kernel output", o.shape, o.dtype)


# revision 9
# speedup vs baseline: 1.2915x; 1.2915x over previous
"""Trainium2 Bass kernel for batched multi-head attention.

Problem: B=8, H=8, S=2048, D=64 f32 attention,
  out = softmax(Q K^T / 64**0.25) V  per (b, h).

Sharding: the 64 (b,h) pairs are split 8-per-core across the 8 NeuronCores
(pure data/head parallelism, no collectives).

v2 design (vs v1 baseline at ~315us):
  - The v1 kernel was ACT-bound: all 33.6M softmax exps per core ran on the
    Scalar engine (~260us busy).  v2 splits exp across engines: 5/8 of the
    k-chunk pairs use exact ACT exp, 3/8 use a Schraudolph-style fast exp on
    the (previously idle) Vector engine - one tensor_scalar op computing
    int16(bf16-bits) = scores*A + B, bit-reinterpreted as bf16 weights
    (rel-err ~1e-2 end to end, tolerance 2e-2).
  - QK strip pairs (K=64 contraction) get explicit tile_position row tiling
    so both 64-row matmuls stream concurrently.
  - Issue order software-pipelines one (head, slab) stage: QK of stage i+1
    issues before AV of stage i, so the exp engines never wait on the PE
    FIFO draining AV before producing the next slab's scores.
  - Softmax denominators come from a ones-column in the AV stationary
    (PSUM row 64); per-slab they bounce through DRAM into a [64, 512]
    broadcast and fold into the single normalize tensor_tensor multiply
    (no separate PSUM->SBUF copy).
"""
import sys

sys.path.insert(0, "/opt/trn_rl_repo")

from contextlib import ExitStack

import ml_dtypes
import numpy as np

import concourse.bass as bass
import concourse.tile as tile
from concourse import bacc, mybir
from concourse.bass_utils import run_bass_kernel_spmd

B, H, S, D = 8, 8, 2048, 64
N_CORES = 8
HPC = B * H // N_CORES  # heads per core = 8
SCALE = 1.0 / (D**0.5) ** 0.5  # 1 / 64**0.25
PCHUNK = 128  # k rows per chunk
NCHUNK = S // PCHUNK  # 16
NPAIR = NCHUNK // 2  # 8 chunk pairs per slab
SLAB = 512  # q columns per slab
NSLAB = S // SLAB  # 4
BF16 = mybir.dt.bfloat16
F32 = mybir.dt.float32
I16 = mybir.dt.int16

# Schraudolph fast-exp on DVE: bf16 bits of exp(s*SCALE) ~= s*A1 + B1
LOG2E = 1.4426950408889634
FEXP_C = 0.06  # sawtooth centering, tuned in numerics_sim.py
A1 = 128.0 * LOG2E * SCALE
B1 = 128.0 * (127.0 - FEXP_C)
DVE_PAIRS = (2, 5, 7)  # pairs computed on DVE; rest on ACT

_COMPILED = {}


def build_kernel():
    nc = bacc.Bacc("TRN2", target_bir_lowering=False, debug=False)
    qt = nc.dram_tensor("q_t", [HPC, D, S], BF16, kind="ExternalInput").ap()
    kt = nc.dram_tensor("k_t", [HPC, D, S], BF16, kind="ExternalInput").ap()
    v = nc.dram_tensor("v", [HPC, S, D], BF16, kind="ExternalInput").ap()
    out = nc.dram_tensor("out_t", [HPC, D, S], F32, kind="ExternalOutput").ap()
    # DRAM bounce buffers for the cross-partition softmax-denominator move
    s_dram = nc.dram_tensor("s_scratch", [HPC, NSLAB, SLAB], F32).ap()
    r_dram = nc.dram_tensor("r_scratch", [HPC, NSLAB, SLAB], F32).ap()

    with tile.TileContext(nc) as tc, ExitStack() as ctx:
        qk_pool = ctx.enter_context(tc.tile_pool(name="qk", bufs=2))
        v_pool = ctx.enter_context(tc.tile_pool(name="vp", bufs=2))
        exp_pool = ctx.enter_context(tc.tile_pool(name="exp", bufs=2))
        fin_pool = ctx.enter_context(tc.tile_pool(name="fin", bufs=3))
        small_pool = ctx.enter_context(tc.tile_pool(name="small", bufs=4))
        const_pool = ctx.enter_context(tc.tile_pool(name="const", bufs=1))
        # PSUM: psqk 2 x 2 banks + psav 4 x 1 bank = 8 banks exactly.
        # psav holds 4 stages: po(i) accumulates at stage i+1 and is only
        # read by the normalize multiply at stage i+3 (the denominator's
        # 4-hop DMA chain is spread over the intervening stage slots so no
        # strict-FIFO engine queue ever parks on an unmet dependency).
        psqk_pool = ctx.enter_context(
            tc.tile_pool(name="psqk", bufs=2, space="PSUM")
        )
        psav_pool = ctx.enter_context(
            tc.tile_pool(name="psav", bufs=4, space="PSUM")
        )

        zbias = const_pool.tile([128, 1], F32)
        nc.vector.memset(zbias[:], 0.0)
        # warm the ACT exp table at t=0 so its ~2.7us load overlaps the first
        # input DMAs instead of delaying the first real exp
        warm = const_pool.tile([128, 1], F32)
        nc.scalar.activation(
            warm[:],
            zbias[:],
            mybir.ActivationFunctionType.Exp,
            bias=zbias[:],
            scale=1.0,
        )

        head_tiles = {}

        def load_head(h):
            qt_sb = qk_pool.tile([2 * D, S], BF16, tag="qt")
            kt_sb = qk_pool.tile([2 * D, S], BF16, tag="kt")
            HS = S // 2
            for piece in range(2):
                cols = slice(piece * HS, (piece + 1) * HS)
                nc.sync.dma_start(kt_sb[0:D, cols], kt[h][:, cols])
                nc.sync.dma_start(kt_sb[D : 2 * D, cols], kt[h][:, cols])
                nc.sync.dma_start(qt_sb[0:D, cols], qt[h][:, cols])
                nc.sync.dma_start(qt_sb[D : 2 * D, cols], qt[h][:, cols])
            v_aug = v_pool.tile([PCHUNK, NCHUNK, D + 1], BF16, tag="vaug")
            nc.sync.dma_start(
                v_aug[:, :, 0:D], v[h].rearrange("(c p) d -> p c d", p=PCHUNK)
            )
            nc.vector.memset(v_aug[:, :, D : D + 1], 1.0)
            head_tiles[h] = (qt_sb, kt_sb, v_aug)

        # Per-stage pipeline state: stage i = (head, slab).
        st = {}  # i -> dict(h, s, expT, po, av_done, ...)

        def qk_pair(i, p, prev):
            """Issue QK pair p of stage i, then (interleaved in the PE FIFO)
            two AV chunk matmuls of stage i-1 so the PE never drains."""
            d = st[i]
            h, s = d["h"], d["s"]
            qt_sb, kt_sb, _ = head_tiles[h]
            cols = slice(s * SLAB, (s + 1) * SLAB)
            ps = psqk_pool.tile([PCHUNK, 2, SLAB], F32, tag="psqk")
            for half in range(2):
                c = 2 * p + half
                base = half * D
                nc.tensor.matmul(
                    ps[:, half, :],
                    kt_sb[base : base + D, c * PCHUNK : (c + 1) * PCHUNK],
                    qt_sb[base : base + D, cols],
                    start=True,
                    stop=True,
                    tile_position=(base, 0),
                )
            if prev is not None:
                av_chunks(prev, 2 * p)
            dst = d["expT"][:, 2 * p : 2 * p + 2, :]
            if p in DVE_PAIRS:
                nc.vector.tensor_scalar(
                    dst.bitcast(I16),
                    ps[:],
                    A1,
                    B1,
                    mybir.AluOpType.mult,
                    mybir.AluOpType.add,
                )
            else:
                nc.scalar.activation(
                    dst,
                    ps[:],
                    mybir.ActivationFunctionType.Exp,
                    bias=zbias[:],
                    scale=SCALE,
                )

        def av_chunks(i, c0):
            d = st[i]
            h = d["h"]
            _, _, v_aug = head_tiles[h]
            if d["po"] is None:
                po = psav_pool.tile([D + 1, SLAB], F32, tag="psav")
                d["po"] = po
            for c in (c0, c0 + 1):
                nc.tensor.matmul(
                    d["po"][:],
                    v_aug[:, c, :],
                    d["expT"][:, c, :],
                    start=(c == 0),
                    stop=(c == NCHUNK - 1),
                )

        def norm_rowcopy(i):
            # PSUM sums row -> SBUF (DMA cannot read PSUM) -> DRAM ->
            # [128, SLAB/128] reload; reciprocal happens next stage slot
            d = st[i]
            h, s = d["h"], d["s"]
            row_sb = small_pool.tile([1, SLAB], F32, tag="rowsb")
            nc.vector.tensor_copy(row_sb[:], d["po"][D : D + 1, :])
            nc.sync.dma_start(s_dram[h, s], row_sb[:])
            sums = small_pool.tile([128, SLAB // 128], F32, tag="sums")
            nc.sync.dma_start(
                sums[:], s_dram[h, s].rearrange("(c p) -> p c", p=128)
            )
            d["sums"] = sums

        def norm_recip(i):
            d = st[i]
            h, s = d["h"], d["s"]
            rnat = small_pool.tile([128, SLAB // 128], F32, tag="rnat")
            nc.vector.reciprocal(rnat[:], d["sums"][:])
            nc.sync.dma_start(
                r_dram[h, s].rearrange("(c p) -> p c", p=128), rnat[:]
            )
            r_bc = small_pool.tile([D, SLAB], F32, tag="rbc")
            nc.sync.dma_start(
                r_bc[:],
                bass.AP(
                    r_dram.tensor, (h * NSLAB + s) * SLAB, [[0, D], [1, SLAB]]
                ),
            )
            d["r_bc"] = r_bc

        def norm_mult(i):
            d = st[i]
            h, s = d["h"], d["s"]
            o_fin = fin_pool.tile([D, SLAB], F32, tag="ofin")
            nc.vector.tensor_tensor(
                o_fin[:], d["po"][0:D, :], d["r_bc"][:], op=mybir.AluOpType.mult
            )
            nc.sync.dma_start(out[h][:, s * SLAB : (s + 1) * SLAB], o_fin[:])
            del st[i]

        stages = [(h, s) for h in range(HPC) for s in range(NSLAB)]
        n = len(stages)
        for i in range(n + 4):
            if i < n:
                h, s = stages[i]
                if s == 0:
                    load_head(h)
                expT = exp_pool.tile([PCHUNK, NCHUNK, SLAB], BF16, tag="expT")
                st[i] = {"h": h, "s": s, "po": None, "expT": expT}
                for p in range(NPAIR):
                    qk_pair(i, p, i - 1 if i >= 1 else None)
            elif i == n:
                # drain the last stage's AV matmuls
                for c0 in range(0, NCHUNK, 2):
                    av_chunks(n - 1, c0)
            if i - 1 >= 0 and i - 1 in st and st[i - 1]["po"] is not None:
                norm_rowcopy(i - 1)
            if i - 2 >= 0 and i - 2 in st and "sums" in st[i - 2]:
                norm_recip(i - 2)
            if i - 3 >= 0 and i - 3 in st and "r_bc" in st[i - 3]:
                norm_mult(i - 3)
    nc.compile()
    return nc


def _get_compiled():
    if "nc" not in _COMPILED:
        _COMPILED["nc"] = build_kernel()
    return _COMPILED["nc"]


def kernel(query, key, value, _want_results=False):
    nc = _get_compiled()
    q = np.asarray(query).reshape(B * H, S, D)
    k = np.asarray(key).reshape(B * H, S, D)
    v = np.asarray(value).reshape(B * H, S, D)
    in_maps = []
    for c in range(N_CORES):
        sl = slice(c * HPC, (c + 1) * HPC)
        in_maps.append(
            {
                "q_t": np.ascontiguousarray(q[sl].transpose(0, 2, 1)).astype(
                    ml_dtypes.bfloat16
                ),
                "k_t": np.ascontiguousarray(k[sl].transpose(0, 2, 1)).astype(
                    ml_dtypes.bfloat16
                ),
                "v": np.ascontiguousarray(v[sl]).astype(ml_dtypes.bfloat16),
            }
        )
    res = run_bass_kernel_spmd(nc, in_maps, core_ids=list(range(N_CORES)))
    out = np.concatenate(
        [
            res.results[c]["out_t"].transpose(0, 2, 1).reshape(1, HPC, S, D)
            for c in range(N_CORES)
        ],
        axis=0,
    ).reshape(B, H, S, D)
    if _want_results:
        return out, res
    return out


if __name__ == "__main__":
    rng = np.random.default_rng(0)
    q = rng.standard_normal((B, H, S, D), dtype=np.float32)
    k = rng.standard_normal((B, H, S, D), dtype=np.float32)
    v = rng.standard_normal((B, H, S, D), dtype=np.float32)
    o = kernel(q, k, v)
    print("kernel output", o.shape, o.dtype)
